# revision 14
# baseline (speedup 1.0000x reference)
"""ByteEncoder Trainium2 kernel (v2).

Model: h = embed[x]; y = Conv1d(k=4, s=4)(h); y = LN(y)*g+b; xb = y@bW.T+bb;
       h_t = lam*h_{t-1} + (1-lam)*xb_t (LRU scan); out = h@cW.T+cb.

Strategy (8 NeuronCores, data-parallel over (batch, half-sequence)):
  * No collectives: each core prepends a W=128 conv-token warmup window
    (the preceding tokens of the batch row) and runs its scan from zero.
    lam = sigmoid(exp(log_lambda)) <= ~0.83, so the dropped history term
    is bounded by lam^W ~ 1e-12 -- far below fp32 noise.  First-half
    cores mask the warmup scan input with a parity flag, making them
    exact.  All 8 cores are fully independent (no cross-core skew cost).
  * embed+conv fused into 4 LUTs LUT_j[v,o] = sum_d embed[v,d]*conv_w[o,d,j]
    built on-device by GEMM; conv becomes LUT^T @ onehot.  The LUT build
    (phase A) and the conv GEMM (phase B) are interleaved per 256-wide
    column block of conv_w, streamed j-by-j, so the PE pipelines inside
    the 16MB conv_w DMA window.
  * LayerNorm is folded into the b-projection epilogue:
      u = oml * rstd_t * (G - mu_t * S) + c0,
    G = (g*bW)^T @ y_raw, S_o = sum_d g_d bW[o,d], b2_o = sum_d bW[o,d] lnb_d,
    c0 = oml*(b2+bb), oml = 1-lam.  S and b2 come from two column-broadcast
    lhsT matmuls; mu/var from ones-matmul column sums.  No separate
    normalize pass over y.
  * LRU scan on DVE tensor_tensor_scan over [128, 1152] channel-major
    tiles; c-projection consumes the scan output directly as matmul
    weights (h columns W..W+1024).
  * Matmuls run as float32r (full fp32 data, 1 cycle/row at free >= 256).
"""

import sys

sys.path.insert(0, "/opt/trn_rl_repo")

from contextlib import ExitStack

import numpy as np

import concourse.bass as bass
import concourse.tile as tile
from concourse import mybir

B, T, D = 4, 8192, 1024
NCORES = 8
TC = T // 4            # 2048 conv tokens per batch
TPC = TC // 2          # 1024 main conv tokens per core
W = 128                # warmup conv tokens
TW = TPC + W           # 1152 conv tokens processed per core
XW = TW * 4            # 4608 input tokens per core (incl. warmup)
XPC = TPC * 4          # 4096 main input tokens per core
V = 256                # vocab
P = 128
DT = D // P            # 8 d-tiles (also o-tiles)
VT = V // P            # 2 v-tiles
NJ = 4                 # conv taps
CH = 384               # t-chunk width (fits one PSUM bank, >=256 for fp32r)
NCH = TW // CH         # 3
KB = 256               # o-column block for the A/B interleave
NK = D // KB           # 4
QW = XW // 4           # 1152 x-positions per broadcast quarter

F32 = mybir.dt.float32
F32R = mybir.dt.float32r
I32 = mybir.dt.int32
AF = mybir.ActivationFunctionType
OP = mybir.AluOpType

LN_EPS = 1e-5

# D split into PSUM-bank-sized chunks for negS/b2/F phases
DCH = [(0, 384), (384, 768), (768, 1024)]


def _vec_view(dram_ap):
    """[D] dram vector -> [128, 8] view (partition p, free dt; d = dt*128+p)."""
    return dram_ap.rearrange("(dt p) -> p dt", p=P)


def build_nc():
    nc = bass.Bass(trn_type="TRN2", num_devices=NCORES)

    x_f = nc.declare_dram_parameter("x_f", [XW], F32, isOutput=False)
    embedT = nc.declare_dram_parameter("embedT", [D, V], F32, isOutput=False)
    convwT = nc.declare_dram_parameter("convwT", [NJ, D, D], F32, isOutput=False)
    bWT = nc.declare_dram_parameter("bWT", [D, D], F32, isOutput=False)
    cWT = nc.declare_dram_parameter("cWT", [D, D], F32, isOutput=False)
    conv_b = nc.declare_dram_parameter("conv_b", [D], F32, isOutput=False)
    ln_g = nc.declare_dram_parameter("ln_g", [D], F32, isOutput=False)
    ln_b = nc.declare_dram_parameter("ln_b", [D], F32, isOutput=False)
    log_lambda = nc.declare_dram_parameter("log_lambda", [D], F32, isOutput=False)
    bb = nc.declare_dram_parameter("bb", [D], F32, isOutput=False)
    cb = nc.declare_dram_parameter("cb", [D], F32, isOutput=False)
    parity = nc.declare_dram_parameter("parity", [1], F32, isOutput=False)
    out = nc.declare_dram_parameter("out", [TPC, D], F32, isOutput=True)

    rc_dram = nc.dram_tensor("rc_dram", [2, P, D], F32)

    with tile.TileContext(nc) as tc, ExitStack() as ctx, \
            nc.allow_low_precision(reason="float32r matmul operands"):
        _body(ctx, tc, x_f.ap(), embedT.ap(), convwT.ap(), bWT.ap(), cWT.ap(),
              conv_b.ap(), ln_g.ap(), ln_b.ap(), log_lambda.ap(), bb.ap(),
              cb.ap(), parity.ap(), out.ap(), rc_dram.ap())
    _split_excess_waits(nc)
    return nc


def _split_excess_waits(nc, max_waits=1):
    """walrus codegen allows only one sync-wait slot per TPB instruction;
    hoist excess waits onto single-wait NoOps inserted just before the
    instruction on the same engine queue (queue order makes this exact)."""
    cnt = 0
    for f in nc.m.functions:
        for b in f.blocks:
            insts = list(b.instructions)
            out_list = []
            for inst in insts:
                si = inst.sync_info
                waits = list(si.on_wait) if si is not None and si.on_wait else []
                if len(waits) > max_waits:
                    for w in waits[:-max_waits]:
                        nop = mybir.InstNoOp(
                            name=f"waitsplit_{cnt}",
                            sync_info=mybir.SyncInfo(on_wait=[w], on_update=[]),
                        )
                        nop.engine = inst.engine
                        nc.inst_map[nop.name] = nop
                        cnt += 1
                        out_list.append(nop)
                    inst.sync_info = mybir.SyncInfo(
                        on_wait=waits[-max_waits:],
                        on_update=list(si.on_update) if si.on_update else [])
                out_list.append(inst)
            b.instructions = out_list
    return cnt


def _body(ctx, tc, x_f, embedT, convwT, bWT, cWT, conv_b, ln_g, ln_b,
          log_lambda, bb, cb, parity, out, rc_dram):
    nc = tc.nc

    # ---------------- SBUF pools ----------------
    # Big [128, TW] f32 groups (4.5KB/part each slot):
    #   g1: onehot -> u     g2: y      g3: x quarters -> h
    g1 = ctx.enter_context(tc.tile_pool(name="g1", bufs=1))
    g2 = ctx.enter_context(tc.tile_pool(name="g2", bufs=1))
    g3 = ctx.enter_context(tc.tile_pool(name="g3", bufs=1))
    g4 = ctx.enter_context(tc.tile_pool(name="g4", bufs=1))   # bwt [128,1024]
    g5 = ctx.enter_context(tc.tile_pool(name="g5", bufs=1))   # lut -> cwt
    ring = ctx.enter_context(tc.tile_pool(name="ring", bufs=6))   # cw stream
    y2p = ctx.enter_context(tc.tile_pool(name="y2p", bufs=3))     # y^2 / scratch
    bcr = ctx.enter_context(tc.tile_pool(name="bcr", bufs=2))     # lhsT bcast ring
    rows = ctx.enter_context(tc.tile_pool(name="rows", bufs=1))   # stats rows + cb
    small = ctx.enter_context(tc.tile_pool(name="small", bufs=1))
    # PSUM: 6 banks for the long-lived conv/b-proj accumulators, 2 for the rest
    pp6 = ctx.enter_context(tc.tile_pool(name="pp6", bufs=6, space="PSUM"))
    pp2 = ctx.enter_context(tc.tile_pool(name="pp2", bufs=2, space="PSUM"))

    _uid = [0]

    def uname(pfx):
        _uid[0] += 1
        return f"{pfx}_{_uid[0]}"

    # ---------------- phase 0: constants ----------------
    convb_t = small.tile([P, DT], F32, tag="convb")
    g_t = small.tile([P, DT], F32, tag="g")
    lnb_t = small.tile([P, DT], F32, tag="lnb")
    ll_t = small.tile([P, DT], F32, tag="ll")
    bb_t = small.tile([P, DT], F32, tag="bb")
    nc.sync.dma_start(out=convb_t, in_=_vec_view(conv_b))
    nc.sync.dma_start(out=g_t, in_=_vec_view(ln_g))
    nc.sync.dma_start(out=lnb_t, in_=_vec_view(ln_b))
    nc.sync.dma_start(out=ll_t, in_=_vec_view(log_lambda))
    nc.sync.dma_start(out=bb_t, in_=_vec_view(bb))
    parity_sb = small.tile([P, 1], F32, tag="parity")
    nc.sync.dma_start(out=parity_sb, in_=parity.partition_broadcast(P))

    # lam = sigmoid(exp(log_lambda)); oml = 1-lam
    e_t = small.tile([P, DT], F32, tag="e")
    lam_t = small.tile([P, DT], F32, tag="lam")
    oml_t = small.tile([P, DT], F32, tag="oml")
    nc.scalar.activation(out=e_t, in_=ll_t, func=AF.Exp)
    nc.scalar.activation(out=lam_t, in_=e_t, func=AF.Sigmoid)
    nc.vector.tensor_scalar(out=oml_t, in0=lam_t, scalar1=-1.0, scalar2=1.0,
                            op0=OP.mult, op1=OP.add)

    ones_m32 = small.tile([P, P], F32, tag="ones_m32")
    nc.vector.memset(ones_m32, 1.0)
    ones_mat = small.tile([P, P], F32R, tag="ones_mat")
    nc.vector.tensor_copy(out=ones_mat, in_=ones_m32)

    iota_v = small.tile([P, 1], I32, tag="iota_v")
    nc.gpsimd.iota(iota_v, [[0, 1]], base=0, channel_multiplier=1)
    iota_vf = small.tile([P, 1], F32, tag="iota_vf")
    nc.vector.tensor_copy(out=iota_vf, in_=iota_v)
    iota_vf2 = small.tile([P, 1], F32, tag="iota_vf2")
    nc.vector.tensor_scalar(out=iota_vf2, in0=iota_vf, scalar1=float(P),
                            scalar2=None, op0=OP.add)

    eps_sb = small.tile([P, 1], F32, tag="eps")
    nc.vector.memset(eps_sb, LN_EPS)

    # ---------------- embedT tile ----------------
    et = small.tile([P, DT, V], F32R, tag="et")
    nc.sync.dma_start(
        out=et, in_=embedT.bitcast(F32R).rearrange("(dt p) v -> p dt v", p=P))

    # ---------------- x broadcast + onehot ----------------
    xq = []
    for q in range(4):
        t_ = g3.tile([P, QW], F32, tag=f"g3_{q}", name=uname("xq"))
        nc.sync.dma_start(out=t_, in_=x_f[q * QW:(q + 1) * QW].partition_broadcast(P))
        xq.append(t_)

    QT = QW // NJ  # 288 conv tokens per quarter
    oh = {}
    for j in range(NJ):
        for vt in range(VT):
            i8 = j * VT + vt
            o_t = g1.tile([P, TW], F32R, tag=f"g1_{i8}", name=uname("oh"))
            oh[(j, vt)] = o_t
            iv = iota_vf if vt == 0 else iota_vf2
            eng = nc.vector if i8 < 5 else nc.gpsimd
            for q in range(4):
                xv = xq[q].rearrange("p (t j) -> p t j", j=NJ)[:, :, j]
                eng.tensor_scalar(out=o_t[:, q * QT:(q + 1) * QT],
                                  in0=xv, scalar1=iv, scalar2=None,
                                  op0=OP.is_equal)

    # ---------------- phases A+B interleaved (stream conv_w column blocks) ----
    lut = {}
    for j in range(NJ):
        for vt in range(VT):
            lut[(j, vt)] = g5.tile([P, D], F32R, tag=f"g5_{j * VT + vt}",
                                   name=uname("lut"))
    y = [g2.tile([P, TW], F32R, tag=f"g2_{ot}", name=uname("y"))
         for ot in range(DT)]
    musum = rows.tile([P, TW], F32, tag="musum")
    sqsum = rows.tile([P, TW], F32, tag="sqsum")

    cwT_r = convwT.bitcast(F32R).rearrange("j (dt p) o -> j dt p o", p=P)

    for k in range(NK):
        osl = slice(k * KB, (k + 1) * KB)
        ots = (2 * k, 2 * k + 1)
        # conv psums for this k-block: live across all 4 j-steps
        psy = {}
        for ot in ots:
            for c in range(NCH):
                psy[(ot, c)] = pp6.tile([P, CH], F32, tag="psy",
                                        name=uname(f"psy{k}"))
        for j in range(NJ):
            # A: accumulate LUT_j columns [k*KB, (k+1)*KB) over d
            psA = [pp2.tile([P, CH], F32, tag="pm", name=uname(f"psA{k}{j}"))
                   for _ in range(VT)]
            for dt_ in range(DT):
                cw = ring.tile([P, KB], F32R, tag="cw", name=uname("cw"))
                nc.sync.dma_start(out=cw, in_=cwT_r[j, dt_][:, osl])
                for vt in range(VT):
                    nc.tensor.matmul(psA[vt][:, 0:KB],
                                     et[:, dt_, vt * P:(vt + 1) * P], cw,
                                     start=(dt_ == 0), stop=(dt_ == DT - 1))
            for vt in range(VT):
                nc.scalar.activation(out=lut[(j, vt)][:, osl],
                                     in_=psA[vt][:, 0:KB], func=AF.Copy)
            # B: add this j's contribution to y rows of this k-block
            for ot in ots:
                for c in range(NCH):
                    csl = slice(c * CH, (c + 1) * CH)
                    for vt in range(VT):
                        nc.tensor.matmul(
                            psy[(ot, c)],
                            lut[(j, vt)][:, ot * P:(ot + 1) * P],
                            oh[(j, vt)][:, csl],
                            start=(j == 0 and vt == 0),
                            stop=(j == NJ - 1 and vt == VT - 1))
        # epilogue: y = psum + conv_b
        for ot in ots:
            for c in range(NCH):
                csl = slice(c * CH, (c + 1) * CH)
                nc.scalar.activation(out=y[ot][:, csl], in_=psy[(ot, c)],
                                     func=AF.Identity,
                                     bias=convb_t[:, ot:ot + 1], scale=1.0)
        # stats for this k-block's two o-tiles (raw y)
        for c in range(NCH):
            csl = slice(c * CH, (c + 1) * CH)
            ps_ = pp2.tile([P, CH], F32, tag="pm", name=uname(f"pss{k}"))
            for i, ot in enumerate(ots):
                nc.tensor.matmul(ps_, ones_mat, y[ot][:, csl],
                                 start=(i == 0), stop=(i == len(ots) - 1))
            if k == 0:
                nc.vector.tensor_copy(out=musum[:, csl], in_=ps_)
            else:
                nc.vector.tensor_add(out=musum[:, csl], in0=musum[:, csl],
                                     in1=ps_)
        for c in range(NCH):
            csl = slice(c * CH, (c + 1) * CH)
            ps_ = pp2.tile([P, CH], F32, tag="pm", name=uname(f"psq{k}"))
            for i, ot in enumerate(ots):
                y2 = y2p.tile([P, CH], F32R, tag="y2", name=uname("y2"))
                nc.scalar.activation(out=y2, in_=y[ot][:, csl].bitcast(F32),
                                     func=AF.Square)
                nc.tensor.matmul(ps_, ones_mat, y2,
                                 start=(i == 0), stop=(i == len(ots) - 1))
            if k == 0:
                nc.vector.tensor_copy(out=sqsum[:, csl], in_=ps_)
            else:
                nc.vector.tensor_add(out=sqsum[:, csl], in0=sqsum[:, csl],
                                     in1=ps_)
        # fold ln_g into y in place (after stats read raw y)
        for i, ot in enumerate(ots):
            eng = nc.vector if i % 2 == 0 else nc.gpsimd
            eng.tensor_scalar(out=y[ot], in0=y[ot][:, :].bitcast(F32),
                              scalar1=g_t[:, ot:ot + 1], scalar2=None,
                              op0=OP.mult)

    # ---------------- bWT arrives; negS/b2 row matmuls ----------------
    bwt = []
    for dt_ in range(DT):
        t_ = g4.tile([P, D], F32R, tag=f"g4_{dt_}", name=uname("bwt"))
        bwt.append(t_)
        nc.sync.dma_start(
            out=t_, in_=bWT.bitcast(F32R).rearrange("(dt p) o -> dt p o", p=P)[dt_])

    # negS_row[o] = -sum_d g[d]*bW[o,d]; b2_row[o] = sum_d lnb[d]*bW[o,d]
    for row_i, (vec_t, scl) in enumerate(((g_t, -1.0), (lnb_t, 1.0))):
        for (c0_, c1_) in DCH:
            cw_ = c1_ - c0_
            ps_ = pp2.tile([P, CH], F32, tag="pm", name=uname("psrow"))
            for dt_ in range(DT):
                vbc = bcr.tile([P, P], F32R, tag="vbc", name=uname("vbc"))
                nc.gpsimd.tensor_copy(
                    out=vbc, in_=vec_t[:, dt_:dt_ + 1].broadcast_to((P, P)))
                nc.tensor.matmul(ps_[:, 0:cw_], vbc, bwt[dt_][:, c0_:c1_],
                                 start=(dt_ == 0), stop=(dt_ == DT - 1))
            tmp = y2p.tile([P, CH], F32, tag="y2", name=uname("rowtmp"))
            nc.scalar.activation(out=tmp[:, 0:cw_], in_=ps_[:, 0:cw_],
                                 func=AF.Copy, scale=scl)
            nc.sync.dma_start(out=rc_dram[row_i][:, c0_:c1_],
                              in_=tmp[:, 0:cw_])

    negS_c = small.tile([P, DT], F32, tag="negS")
    b2_c = small.tile([P, DT], F32, tag="b2c")
    nc.sync.dma_start(out=negS_c, in_=_vec_view(rc_dram[0, 0]))
    nc.sync.dma_start(out=b2_c, in_=_vec_view(rc_dram[1, 0]))
    c0_c = small.tile([P, DT], F32, tag="c0c")
    nc.vector.tensor_add(out=c0_c, in0=b2_c, in1=bb_t)
    nc.vector.tensor_mul(out=c0_c, in0=c0_c, in1=oml_t)

    # ---------------- finalize LN stats rows ----------------
    # mu = musum/D (in place); var = sqsum/D - mu^2; rstd = 1/sqrt(var+eps)
    nc.scalar.activation(out=musum, in_=musum, func=AF.Copy, scale=1.0 / D)
    nc.scalar.activation(out=sqsum, in_=sqsum, func=AF.Copy, scale=1.0 / D)
    for c in range(NCH):
        csl = slice(c * CH, (c + 1) * CH)
        mu2 = y2p.tile([P, CH], F32, tag="y2", name=uname("mu2"))
        nc.vector.tensor_mul(out=mu2, in0=musum[:, csl], in1=musum[:, csl])
        nc.vector.tensor_sub(out=sqsum[:, csl], in0=sqsum[:, csl], in1=mu2)
    nc.scalar.activation(out=sqsum, in_=sqsum, func=AF.Sqrt, bias=eps_sb)
    nc.vector.reciprocal(out=sqsum, in_=sqsum)
    mu_row, rstd_row = musum, sqsum

    # ---------------- cWT prefetch (into dead LUT slots) ----------------
    cwt = []
    for dt_ in range(DT):
        t_ = g5.tile([P, D], F32R, tag=f"g5_{dt_}", name=uname("cwt"))
        cwt.append(t_)
        nc.sync.dma_start(
            out=t_, in_=cWT.bitcast(F32R).rearrange("(dt p) o -> dt p o", p=P)[dt_])

    # cb broadcast row for the F epilogue (tiny, needed late)
    cb_bc = rows.tile([P, D], F32, tag="cb")
    nc.sync.dma_start(out=cb_bc, in_=cb.partition_broadcast(P))

    # ---------------- phase D: b-projection + LN epilogue + scan ----------
    u = []
    h_sb = []
    for ot in range(DT):
        u_t = g1.tile([P, TW], F32, tag=f"g1_{ot}", name=uname("u"))
        u.append(u_t)
        for c in range(NCH):
            csl = slice(c * CH, (c + 1) * CH)
            psx = pp6.tile([P, CH], F32, tag="psy", name=uname("psx"))
            for dt_ in range(DT):
                nc.tensor.matmul(psx, bwt[dt_][:, ot * P:(ot + 1) * P],
                                 y[dt_][:, csl],
                                 start=(dt_ == 0), stop=(dt_ == DT - 1))
            # u = oml*rstd*(G - mu*S) + c0, as
            #   e = mu*negS + G ; e *= rstd ; u = e*oml + c0
            # (STT reads PSUM -> DVE only; the rstd multiply is SBUF-only
            # and runs on Pool to keep DVE free for the scans)
            nc.vector.scalar_tensor_tensor(
                out=u_t[:, csl], in0=mu_row[:, csl],
                scalar=negS_c[:, ot:ot + 1], in1=psx,
                op0=OP.mult, op1=OP.add)
            nc.gpsimd.tensor_mul(out=u_t[:, csl], in0=u_t[:, csl],
                                 in1=rstd_row[:, csl])
            nc.scalar.activation(out=u_t[:, csl], in_=u_t[:, csl],
                                 func=AF.Identity, scale=oml_t[:, ot:ot + 1],
                                 bias=c0_c[:, ot:ot + 1])
        # mask warmup columns on first-half cores (parity=0 zeroes them)
        nc.gpsimd.tensor_scalar(out=u_t[:, 0:W], in0=u_t[:, 0:W],
                                scalar1=parity_sb, scalar2=None, op0=OP.mult)
        # LRU scan along t
        h_t = g3.tile([P, TW], F32R, tag=f"g3_{ot}", name=uname("h"))
        h_sb.append(h_t)
        lam_bc = lam_t[:, ot:ot + 1].broadcast_to((P, TW))
        nc.vector.tensor_tensor_scan(out=h_t, data0=lam_bc, data1=u_t,
                                     initial=0.0, op0=OP.mult, op1=OP.add)

    # ---------------- phase F: c-projection + cb, DMA out ----------------
    for tt in range(DT):
        hsl = slice(W + tt * P, W + (tt + 1) * P)
        for (c0_, c1_) in DCH:
            cw_ = c1_ - c0_
            pso = pp6.tile([P, CH], F32, tag="psy", name=uname("pso"))
            for dt_ in range(DT):
                nc.tensor.matmul(pso[:, 0:cw_], h_sb[dt_][:, hsl],
                                 cwt[dt_][:, c0_:c1_],
                                 start=(dt_ == 0), stop=(dt_ == DT - 1))
            ostage = y2p.tile([P, CH], F32, tag="y2", name=uname("ost"))
            nc.vector.scalar_tensor_tensor(
                out=ostage[:, 0:cw_], in0=pso[:, 0:cw_], scalar=1.0,
                in1=cb_bc[:, c0_:c1_], op0=OP.mult, op1=OP.add)
            nc.sync.dma_start(out=out[tt * P:(tt + 1) * P, c0_:c1_],
                              in_=ostage[:, 0:cw_])


_NC_CACHE = None


def _get_nc():
    global _NC_CACHE
    if _NC_CACHE is None:
        _NC_CACHE = build_nc()
    return _NC_CACHE


def _in_maps(x, embed, conv_w, conv_b, ln_g, ln_b, log_lambda, bW, bb, cW, cb):
    f = lambda a: np.ascontiguousarray(np.asarray(a, dtype=np.float32))
    x = np.asarray(x)
    embedT = f(np.asarray(embed, np.float32).T)                    # [D, V]
    convwT = f(np.asarray(conv_w, np.float32).transpose(2, 1, 0))  # [j, d, o]
    bWT = f(np.asarray(bW, np.float32).T)                          # [d, o]
    cWT = f(np.asarray(cW, np.float32).T)                          # [d, o]
    shared = dict(embedT=embedT, convwT=convwT, bWT=bWT, cWT=cWT,
                  conv_b=f(conv_b), ln_g=f(ln_g), ln_b=f(ln_b),
                  log_lambda=f(log_lambda), bb=f(bb), cb=f(cb))
    XWARM = W * 4
    maps = []
    for c in range(NCORES):
        b, h = c // 2, c % 2
        s = h * XPC
        if s >= XWARM:
            xi = x[b, s - XWARM:s + XPC]
        else:
            # warmup slice is masked on-core (parity=0); any valid tokens do
            xi = np.concatenate([x[b, 0:XWARM], x[b, s:s + XPC]])
        maps.append(dict(x_f=np.ascontiguousarray(xi.astype(np.float32)),
                         parity=np.array([float(h)], np.float32),
                         **shared))
    return maps


def _unshard(results):
    out = np.empty((B, TC, D), np.float32)
    for c in range(NCORES):
        b, h = c // 2, c % 2
        out[b, h * TPC:(h + 1) * TPC, :] = results[c]["out"]
    return out


def run(trace=False, **inputs):
    from concourse.bass_utils import run_bass_kernel_spmd
    nc = _get_nc()
    maps = _in_maps(**inputs)
    res = run_bass_kernel_spmd(nc, maps, list(range(NCORES)), trace=trace)
    return _unshard(res.results), res


def kernel(**inputs):
    out, _ = run(trace=False, **inputs)
    return out
